# revision 4
# baseline (speedup 1.0000x reference)
"""2-layer GAT (graph attention) on 8 Trainium2 NeuronCores.

Strategy (dst-partitioned graph parallelism):
  - Sort edges by dst. Core c owns dst nodes [c*6250, (c+1)*6250) and the
    contiguous slice of dst-sorted edges targeting them (~100k edges/core).
  - Per layer: each core computes feat_ext = h_blk @ [W | W@AL | W@AR] for its
    own 6250-node block (one matmul), then an AllGather builds the full
    [50176, 260] (feat|el) and [50176, 4] (er) node tables on every core.
  - Edge aggregation per core runs over 49 "windows" of 128 dst nodes. Each
    window processes T_w fixed tiles of 128 edges:
      * indirect-DMA gather of [feat|el] rows by src and er rows by dst
      * logits = el_src + er_dst -> leaky_relu -> exp   (no segment-max:
        logits are O(10), exp is safe in fp32; softmax normalizer is exact)
      * rhs = [exp * feat_src | exp]  (the extra 4 cols accumulate the
        softmax denominator per head)
      * PSUM[128 dst, 260] += S_tile.T @ rhs  where S is the host-built
        edge->dst-slot one-hot; padding edges have all-zero S rows.
      * finalize: rst = PSUM[:, :256] / max(s, 1e-9), + bias, ELU.
  - Layer 2 repeats with h1; lhsT for its matmul comes from PE-transposed
    h1 tiles. Final per-core [6250, 256] blocks are assembled on host into
    the reference's (4, 50000, 64) stacked-heads output.
"""

import sys

for _p in ("/opt/trn_rl_repo",):
    if _p not in sys.path:
        sys.path.insert(0, _p)

import numpy as np

import concourse.bass as bass
import concourse.tile as tile
from concourse import bacc, mybir
from concourse.bass import IndirectOffsetOnAxis
from concourse.bass_utils import run_bass_kernel_spmd
from concourse.masks import make_identity

N_NODES = 50000
N_EDGES = 800000
F_IN = 256
HID = 64
HEADS = 4
HD = HEADS * HID  # 256
NEG_SLOPE = 0.2
NCORES = 8
P = 128
B = N_NODES // NCORES  # 6250 real nodes per core block
NW = 49  # windows of 128 dst per core
BP = NW * P  # 6272 padded block rows
EXT = HD + 2 * HEADS  # 264: feat | el | er
MC = HD + HEADS  # 260: feat | el  (gathered row)

FDT = mybir.dt.float32
IDT = mybir.dt.int32

LAST_TIMING = {}


def _host_prep(x, src, dst, Ws):
    """Build per-core input maps. Ws = (W0,al0,ar0,b0,W1,al1,ar1,b1)."""
    W0, al0, ar0, b0, W1, al1, ar1, b1 = Ws

    def ext_w(W, al, ar):
        A_l = np.zeros((HD, HEADS), np.float32)
        A_r = np.zeros((HD, HEADS), np.float32)
        for h in range(HEADS):
            A_l[h * HID:(h + 1) * HID, h] = al[h]
            A_r[h * HID:(h + 1) * HID, h] = ar[h]
        return np.concatenate([W, W @ A_l, W @ A_r], axis=1).astype(np.float32)

    W0e, W1e = ext_w(W0, al0, ar0), ext_w(W1, al1, ar1)

    order = np.argsort(dst, kind="stable")
    src_s = src[order].astype(np.int64)
    dst_s = dst[order].astype(np.int64)

    # table row remap: node n -> (n // B) * BP + n % B
    def tr(n):
        return ((n // B) * BP + (n % B)).astype(np.int32)

    # per (core, window) edge counts -> global max tiles per window
    edge_block = np.searchsorted(dst_s, np.arange(0, N_NODES + 1, 1))  # cum counts
    win_edges = []
    for c in range(NCORES):
        for w in range(NW):
            d0 = c * B + w * P
            d1 = min(c * B + min((w + 1) * P, B), (c + 1) * B)
            if d0 >= (c + 1) * B:
                win_edges.append(0)
                continue
            win_edges.append(int(edge_block[d1] - edge_block[d0]))
    T_w = max(1, int(np.max([(e + P - 1) // P for e in win_edges])))
    NT = NW * T_w

    in_maps = []
    for c in range(NCORES):
        xb = np.zeros((BP, F_IN), np.float32)
        xb[:B] = x[c * B:(c + 1) * B]
        xT = np.ascontiguousarray(xb.T)  # [256, BP]

        srcI = np.zeros((P, NT), np.int32)
        dstI = np.zeros((P, NT), np.int32)
        S = np.zeros((NT * P, P), np.float32)
        for w in range(NW):
            d0 = c * B + w * P
            d1 = min(c * B + min((w + 1) * P, B), (c + 1) * B)
            e0, e1 = (int(edge_block[d0]), int(edge_block[d1])) if d0 < (c + 1) * B else (0, 0)
            ne = e1 - e0
            es = src_s[e0:e1]
            ed = dst_s[e0:e1]
            dloc = (ed - d0).astype(np.int64)  # 0..127
            for t in range((ne + P - 1) // P):
                ti = w * T_w + t
                lo, hi = t * P, min((t + 1) * P, ne)
                n = hi - lo
                srcI[:n, ti] = tr(es[lo:hi])
                dstI[:n, ti] = tr(ed[lo:hi])
                S[ti * P + np.arange(n), dloc[lo:hi]] = 1.0
        in_maps.append({
            "xT": xT,
            "W0e": W0e, "W1e": W1e,
            "b0f": np.tile(b0.reshape(1, HD), (P, 1)).astype(np.float32),
            "b1f": np.tile(b1.reshape(1, HD), (P, 1)).astype(np.float32),
            "srcI": srcI, "dstI": dstI, "S": S,
        })
    return in_maps, T_w


def _build_nc(T_w):
    NT = NW * T_w
    nc = bacc.Bacc("TRN2", target_bir_lowering=False, debug=False,
                   num_devices=NCORES)
    xT = nc.dram_tensor("xT", [F_IN, BP], FDT, kind="ExternalInput").ap()
    W0e = nc.dram_tensor("W0e", [F_IN, EXT], FDT, kind="ExternalInput").ap()
    W1e = nc.dram_tensor("W1e", [HD, EXT], FDT, kind="ExternalInput").ap()
    b0f = nc.dram_tensor("b0f", [P, HD], FDT, kind="ExternalInput").ap()
    b1f = nc.dram_tensor("b1f", [P, HD], FDT, kind="ExternalInput").ap()
    srcI = nc.dram_tensor("srcI", [P, NT], IDT, kind="ExternalInput").ap()
    dstI = nc.dram_tensor("dstI", [P, NT], IDT, kind="ExternalInput").ap()
    S_d = nc.dram_tensor("S", [NT * P, P], FDT, kind="ExternalInput").ap()
    out = nc.dram_tensor("out", [BP, HD], FDT, kind="ExternalOutput").ap()

    # internal DRAM
    h1 = nc.dram_tensor("h1", [BP, HD], FDT).ap()
    loc_m = [nc.dram_tensor(f"locm{l}", [BP, MC], FDT).ap() for l in range(2)]
    loc_e = [nc.dram_tensor(f"loce{l}", [BP, HEADS], FDT).ap() for l in range(2)]
    T_m = [nc.dram_tensor(f"Tm{l}", [NCORES * BP, MC], FDT,
                          addr_space="Shared").ap() for l in range(2)]
    T_e = [nc.dram_tensor(f"Te{l}", [NCORES * BP, HEADS], FDT,
                          addr_space="Shared").ap() for l in range(2)]

    AX = mybir.AluOpType

    with tile.TileContext(nc) as tc:
        with (
            tc.tile_pool(name="const", bufs=1) as cpool,
            tc.tile_pool(name="wext", bufs=1) as wpool,
            tc.tile_pool(name="lhsT", bufs=3) as lpool,
            tc.tile_pool(name="feat", bufs=3) as fpool,
            tc.tile_pool(name="gwin", bufs=2) as gpool,
            tc.tile_pool(name="stile", bufs=2 * T_w) as spool,
            tc.tile_pool(name="small", bufs=4) as epool,
            tc.tile_pool(name="hout", bufs=3) as hpool,
            tc.tile_pool(name="psA", bufs=2, space="PSUM") as psA,
            tc.tile_pool(name="psW", bufs=2, space="PSUM") as psW,
            tc.tile_pool(name="psT", bufs=2, space="PSUM") as psT,
        ):
            ident = cpool.tile([P, P], FDT)
            make_identity(nc, ident[:])
            srcI_sb = cpool.tile([P, NT], IDT)
            nc.sync.dma_start(srcI_sb[:], srcI[:, :])
            dstI_sb = cpool.tile([P, NT], IDT)
            nc.sync.dma_start(dstI_sb[:], dstI[:, :])
            b_sb = [cpool.tile([P, HD], FDT, name=f"bias{i}", tag=f"bias{i}")
                    for i in range(2)]
            nc.sync.dma_start(b_sb[0][:], b0f[:, :])
            nc.sync.dma_start(b_sb[1][:], b1f[:, :])

            for layer in range(2):
                We_dram = W0e if layer == 0 else W1e
                We = [wpool.tile([P, EXT], FDT, name=f"we{layer}k{k}",
                                 tag=f"we{layer}k{k}") for k in range(2)]
                for k in range(2):
                    nc.sync.dma_start(We[k][:], We_dram[k * P:(k + 1) * P, :])

                # ---- phase A: feat_ext = h @ We ----
                for nt in range(NW):
                    ps = psA.tile([P, EXT], FDT, space="PSUM")
                    for k in range(2):
                        lt = lpool.tile([P, P], FDT, tag="lhsT")
                        if layer == 0:
                            nc.sync.dma_start(
                                lt[:], xT[k * P:(k + 1) * P, nt * P:(nt + 1) * P])
                        else:
                            hsrc = lpool.tile([P, P], FDT, tag="hsrc")
                            nc.sync.dma_start(
                                hsrc[:], h1[nt * P:(nt + 1) * P, k * P:(k + 1) * P])
                            pt = psT.tile([P, P], FDT, space="PSUM")
                            nc.tensor.transpose(pt[:], hsrc[:], ident[:])
                            nc.vector.tensor_copy(lt[:], pt[:])
                        nc.tensor.matmul(ps[:], lt[:], We[k][:],
                                         start=(k == 0), stop=(k == 1))
                    fe = fpool.tile([P, EXT], FDT)
                    nc.vector.tensor_copy(fe[:], ps[:])
                    nc.sync.dma_start(loc_m[layer][nt * P:(nt + 1) * P, :],
                                      fe[:, :MC])
                    nc.sync.dma_start(loc_e[layer][nt * P:(nt + 1) * P, :],
                                      fe[:, MC:EXT])

                # ---- phase B: AllGather tables ----
                nc.gpsimd.collective_compute(
                    "AllGather", AX.bypass,
                    replica_groups=[list(range(NCORES))],
                    ins=[loc_m[layer][:, :]], outs=[T_m[layer][:, :]])
                nc.gpsimd.collective_compute(
                    "AllGather", AX.bypass,
                    replica_groups=[list(range(NCORES))],
                    ins=[loc_e[layer][:, :]], outs=[T_e[layer][:, :]])

                # ---- phase C: windows ----
                for w in range(NW):
                    psw = psW.tile([P, MC], FDT, space="PSUM")
                    Gw = gpool.tile([P, T_w * MC], FDT, tag="G")
                    erw = epool.tile([P, T_w * HEADS], FDT, tag="er")
                    lg = epool.tile([P, T_w * HEADS], FDT, tag="lg")
                    tmp = epool.tile([P, T_w * HEADS], FDT, tag="tmp")
                    exw = epool.tile([P, T_w * HEADS], FDT, tag="ex")
                    Sts = []
                    for t in range(T_w):
                        ti = w * T_w + t
                        nc.gpsimd.indirect_dma_start(
                            out=Gw[:, t * MC:(t + 1) * MC], out_offset=None,
                            in_=T_m[layer][:, :],
                            in_offset=IndirectOffsetOnAxis(
                                ap=srcI_sb[:, ti:ti + 1], axis=0))
                        nc.gpsimd.indirect_dma_start(
                            out=erw[:, t * HEADS:(t + 1) * HEADS],
                            out_offset=None,
                            in_=T_e[layer][:, :],
                            in_offset=IndirectOffsetOnAxis(
                                ap=dstI_sb[:, ti:ti + 1], axis=0))
                        St = spool.tile([P, P], FDT, tag="S")
                        nc.sync.dma_start(St[:], S_d[ti * P:(ti + 1) * P, :])
                        Sts.append(St)

                    el_v = Gw[:].rearrange("p (t c) -> p t c", c=MC)[:, :, HD:MC]
                    er_v = erw[:].rearrange("p (t c) -> p t c", c=HEADS)
                    lg_v = lg[:].rearrange("p (t c) -> p t c", c=HEADS)
                    nc.vector.tensor_tensor(lg_v, el_v, er_v, op=AX.add)
                    nc.vector.tensor_scalar(out=tmp[:], in0=lg[:],
                                            scalar1=NEG_SLOPE, scalar2=None,
                                            op0=AX.mult)
                    nc.vector.tensor_tensor(lg[:], lg[:], tmp[:], op=AX.max)
                    nc.scalar.activation(exw[:], lg[:],
                                         mybir.ActivationFunctionType.Exp)
                    # write exp into the el slots of G (extra matmul cols)
                    ex_v = exw[:].rearrange("p (t c) -> p t c", c=HEADS)
                    nc.vector.tensor_copy(el_v, ex_v)

                    for t in range(T_w):
                        gfeat = Gw[:, t * MC:t * MC + HD].rearrange(
                            "p (h d) -> p h d", d=HID)
                        exb = exw[:, t * HEADS:(t + 1) * HEADS][:, :, None] \
                            .to_broadcast([P, HEADS, HID])
                        nc.vector.tensor_tensor(gfeat, gfeat, exb, op=AX.mult)
                        nc.tensor.matmul(psw[:], Sts[t][:],
                                         Gw[:, t * MC:(t + 1) * MC],
                                         start=(t == 0), stop=(t == T_w - 1))

                    sv = epool.tile([P, HEADS], FDT, tag="sv")
                    nc.vector.tensor_scalar(out=sv[:], in0=psw[:, HD:MC],
                                            scalar1=1e-9, scalar2=None,
                                            op0=AX.max)
                    nc.vector.reciprocal(sv[:], sv[:])
                    ht = hpool.tile([P, HD], FDT, tag="ht")
                    em = hpool.tile([P, HD], FDT, tag="em")
                    svb = sv[:][:, :, None].to_broadcast([P, HEADS, HID])
                    nc.vector.tensor_tensor(
                        ht[:].rearrange("p (h d) -> p h d", d=HID),
                        psw[:, :HD].rearrange("p (h d) -> p h d", d=HID),
                        svb, op=AX.mult)
                    nc.vector.tensor_tensor(ht[:], ht[:], b_sb[layer][:],
                                            op=AX.add)
                    # ELU(v) = max(v,0) + exp(min(v,0)) - 1
                    nc.vector.tensor_scalar(out=em[:], in0=ht[:], scalar1=0.0,
                                            scalar2=None, op0=AX.min)
                    nc.scalar.activation(em[:], em[:],
                                         mybir.ActivationFunctionType.Exp)
                    nc.vector.tensor_scalar(out=ht[:], in0=ht[:], scalar1=0.0,
                                            scalar2=None, op0=AX.max)
                    nc.vector.tensor_tensor(ht[:], ht[:], em[:], op=AX.add)
                    nc.vector.tensor_scalar(out=ht[:], in0=ht[:], scalar1=-1.0,
                                            scalar2=None, op0=AX.add)
                    dst_dram = h1 if layer == 0 else out
                    nc.sync.dma_start(dst_dram[w * P:(w + 1) * P, :], ht[:])

    nc.compile()
    return nc


_CACHE = {}


def kernel(x, src, dst, W0, a_l0, a_r0, b0, W1, a_l1, a_r1, b1):
    import time
    x = np.asarray(x, np.float32)
    src = np.asarray(src, np.int64)
    dst = np.asarray(dst, np.int64)
    Ws = tuple(np.asarray(a, np.float32)
               for a in (W0, a_l0, a_r0, b0, W1, a_l1, a_r1, b1))

    in_maps, T_w = _host_prep(x, src, dst, Ws)
    if T_w not in _CACHE:
        _CACHE[T_w] = _build_nc(T_w)
    nc = _CACHE[T_w]

    t0 = time.perf_counter()
    res = run_bass_kernel_spmd(nc, in_maps, core_ids=list(range(NCORES)))
    t1 = time.perf_counter()
    LAST_TIMING["wall_s"] = t1 - t0

    full = np.concatenate([res.results[c]["out"][:B] for c in range(NCORES)],
                          axis=0)  # [50000, 256]
    return np.stack([full[:, h * HID:(h + 1) * HID] for h in range(HEADS)],
                    axis=0)  # (4, 50000, 64)


if __name__ == "__main__":
    rng = np.random.default_rng(0)
    pass
